# revision 21
# baseline (speedup 1.0000x reference)
# Trainium2 Bass kernel for nn_JustMPNN (segment-mean + 4-layer MLP).
#
# Math: per polymer p, mean_p = mean of its monomer rows (counts cycle 2,3,4);
#       combined = mean + solvent;  out = relu-MLP(combined) @ w4 + b4.
#
# Strategy (data-parallel over polymers, 8 cores), fp16 throughout:
#   - Monomers pre-scaled by 1/count on host, cast fp16 (2 B/elt HBM).
#   - chunk = 42 polymers = 14 triples = 126 monomer rows exactly; one
#     0/1 scatter matrix S [126,42] shared by all chunks. matmul(lhsT=
#     x_chunk[:, dslice], rhs=S) gives meanT[d, p] in PSUM, transposed
#     orientation, f32 accumulation.
#   - Solvent bypasses the PE: host packs solvT [d, p] tiles fp16; one
#     DVE tensor_add per D-chunk fuses the PSUM->SBUF copy with the
#     solvent add, emitting combT fp16.
#   - MLP all-fp16 (1 cyc/row), ACT relu+bias straight out of PSUM with
#     fp16 output. Final layer emits outT [7, tile] via lhsT=w4 (7-col
#     weight load); DVE adds b4 during PSUM->SBUF; one [7, 12600] store
#     at the end; host transposes.
#   - Accuracy (HW-verified + bit-exact host sim): scale-relative absmax
#     ~7e-4, l2rel ~5.4e-4 -- ~28x under the 2e-2 gate.
#
# Host-side packing re-lays every stream so each per-tile DMA is a
# [partitions, contiguous-bytes] transfer (descriptor-efficient).

import os
import sys

import numpy as np

# ---------------------------------------------------------------- constants
P_TOT = 100000
D = 300
H = 128
OUT = 7
N_MONO = 299999

CHUNK_P = 42           # polymers per chunk (14 triples)
CHUNK_M = 126          # monomer rows per chunk
TILE_CHUNKS = 12       # chunks per MLP tile
TILE_P = CHUNK_P * TILE_CHUNKS  # 504

N_CORES = 8
CORE_P = 12501                  # polymers per core (cores 0-6); core 7: 12493
N_CHUNKS = 298                  # ceil(12501/42) -> 12516 padded polys
P_PAD = N_CHUNKS * CHUNK_P      # 12516
N_FULL_TILES = 24               # 24*504 = 12096
PARTIAL_CHUNKS = N_CHUNKS - N_FULL_TILES * TILE_CHUNKS  # 10 chunks = 420
N_TILES = N_FULL_TILES + 1      # 25
P_SLOTS = N_TILES * TILE_P      # 12600 (uniform 504-wide out/solv slots)
M_PAD = N_CHUNKS * CHUNK_M      # 37548

DCH = (0, 128, 256)    # D-chunk offsets
DCWS = (128, 128, 44)  # D-chunk widths


def _import_concourse():
    for p in ("/opt/trn_rl_repo", "/root/.axon_site/_ro/trn_rl_repo"):
        if os.path.isdir(p) and p not in sys.path:
            sys.path.insert(0, p)


def build_smat(dtype, val):
    """S [CHUNK_M, CHUNK_P]: S[r,p] = val for monomer rows of polymer p."""
    s = np.zeros((CHUNK_M, CHUNK_P), dtype=np.float32)
    r = 0
    for p in range(CHUNK_P):
        cnt = 2 + (p % 3)
        s[r: r + cnt, p] = val
        r += cnt
    assert r == CHUNK_M
    return s.astype(dtype)


def emit_program(tc, aps, n_full_tiles, partial_chunks):
    from contextlib import ExitStack

    import concourse.mybir as mybir

    nc = tc.nc
    f32 = mybir.dt.float32
    fp16 = mybir.dt.float16
    Relu = mybir.ActivationFunctionType.Relu
    Add = mybir.AluOpType.add
    Bypass = mybir.AluOpType.bypass

    n_tiles = n_full_tiles + (1 if partial_chunks else 0)

    with ExitStack() as ctx:
        consts = ctx.enter_context(tc.tile_pool(name="consts", bufs=1))
        xpool = ctx.enter_context(tc.tile_pool(name="xp", bufs=3))
        svpool = ctx.enter_context(tc.tile_pool(name="svp", bufs=2))
        cpool = ctx.enter_context(tc.tile_pool(name="cp", bufs=2))
        hpool = ctx.enter_context(tc.tile_pool(name="hp", bufs=2))
        opool = ctx.enter_context(tc.tile_pool(name="op", bufs=1))
        ps1 = ctx.enter_context(tc.tile_pool(name="ps1", bufs=1, space="PSUM"))
        psh = ctx.enter_context(tc.tile_pool(name="psh", bufs=2, space="PSUM"))
        ps4 = ctx.enter_context(tc.tile_pool(name="ps4", bufs=2, space="PSUM"))

        # ---- tile-0 data first (the critical path), then constants
        xhi0 = xpool.tile([CHUNK_M, TILE_CHUNKS * D], fp16, tag="xhi",
                          name="xhi_0")
        nc.sync.dma_start(xhi0[:], aps["xhi"][:, 0: TILE_CHUNKS * D])
        svt0 = svpool.tile([128, 3 * TILE_P], fp16, tag="svt", name="svt_0")
        nc.sync.dma_start(svt0[:], aps["svt"][0])

        s_hi = consts.tile([CHUNK_M, CHUNK_P], fp16)
        nc.sync.dma_start(s_hi[:], aps["s_hi"])

        w1 = consts.tile([128, 3, H], fp16)
        nc.sync.dma_start(w1[:], aps["w1p"].rearrange("j k m -> k j m"))
        w2 = consts.tile([128, H], fp16)
        nc.sync.dma_start(w2[:], aps["w2"])
        w3 = consts.tile([128, H], fp16)
        nc.sync.dma_start(w3[:], aps["w3"])
        w4 = consts.tile([128, OUT], fp16)
        nc.sync.dma_start(w4[:], aps["w4"])
        b1 = consts.tile([128, 1], f32)
        nc.sync.dma_start(b1[:], aps["b1"])
        b2 = consts.tile([128, 1], f32)
        nc.sync.dma_start(b2[:], aps["b2"])
        b3 = consts.tile([128, 1], f32)
        nc.sync.dma_start(b3[:], aps["b3"])
        b4 = consts.tile([OUT, 1], f32)
        nc.sync.dma_start(b4[:], aps["b4"])

        # whole core's output stays in SBUF as outT [7, P_SLOTS]
        out_sb = opool.tile([OUT, P_SLOTS], f32)

        xhi_d = aps["xhi"]
        sv_d = aps["svt"]

        # Tile t's stage-1 runs j-outer so each D-chunk's DVE add (psum +
        # solvT -> combT fp16) fires as soon as that psum completes; tile
        # t-1's MLP layers are woven between stage-1 matmuls at fixed flat
        # indices so the PE never idles long enough to re-throttle.
        def mlp_steps(t, comb_fn, pw):
            state = {}

            def lyr(li, rhs_fn, whi, bias, nk=1):
                def go():
                    rhs = rhs_fn()
                    ph = psh.tile([128, TILE_P], f32, tag="psh",
                                  name=f"ph_{t}_{li}")
                    for k in range(nk):
                        kw = DCWS[k] if nk == 3 else 128
                        lh = whi[0:kw, k, :] if nk == 3 else whi[:]
                        rh = (rhs[0:kw, k * TILE_P: k * TILE_P + pw]
                              if nk == 3 else rhs[:, 0:pw])
                        nc.tensor.matmul(ph[:, 0:pw], lhsT=lh, rhs=rh,
                                         start=(k == 0), stop=(k == nk - 1))
                    hh = hpool.tile([128, TILE_P], fp16, tag=f"hh{li}",
                                    name=f"hh_{t}_{li}")
                    nc.scalar.activation(hh[:, 0:pw], ph[:, 0:pw], Relu,
                                         bias=bias[:, 0:1])
                    state[li] = hh
                return go

            def l4():
                h3 = state[3]
                po = ps4.tile([OUT, TILE_P], f32, tag="ps4", name=f"po_{t}")
                nc.tensor.matmul(po[:, 0:pw], lhsT=w4[:, 0:OUT],
                                 rhs=h3[:, 0:pw], start=True, stop=True)
                nc.vector.tensor_scalar(
                    out=out_sb[:, t * TILE_P: t * TILE_P + pw],
                    in0=po[:, 0:pw], scalar1=b4[:, 0:1], op0=Add,
                    scalar2=None, op1=Bypass)

            return [lyr(1, comb_fn, w1, b1, nk=3),
                    lyr(2, lambda: state[1], w2, b2),
                    lyr(3, lambda: state[2], w3, b3),
                    l4]

        # Two-pass MLP pipeline: tile p's L1/L2 are woven into stage-1 of
        # pass p+1 and L3/L4 into pass p+2, so each layer's ACT latency
        # hides under a full stage-1 matmul stream and the PE stays dense.
        prev1 = None  # [L1, L2, L3, L4] thunks of tile t-1
        prev2 = None  # [L3, L4] thunks of tile t-2
        for t in range(n_tiles):
            nch = TILE_CHUNKS if t < n_full_tiles else partial_chunks
            pw = nch * CHUNK_P

            # ---- loads (per-partition contiguous); tile 0 preloaded
            if t == 0:
                xhi, svt = xhi0, svt0
            else:
                xhi = xpool.tile([CHUNK_M, TILE_CHUNKS * D], fp16, tag="xhi",
                                 name=f"xhi_{t}")
                nc.sync.dma_start(
                    xhi[:, 0: nch * D],
                    xhi_d[:, t * TILE_CHUNKS * D:
                          t * TILE_CHUNKS * D + nch * D])
                svt = svpool.tile([128, 3 * TILE_P], fp16, tag="svt",
                                  name=f"svt_{t}")
                nc.sync.dma_start(svt[:], sv_d[t])

            pst = [ps1.tile([DCWS[j], TILE_P], f32, tag=f"s1_{j}",
                            name=f"s1_{j}_{t}") for j in range(3)]
            comb = cpool.tile([128, 3 * TILE_P], fp16, tag="comb",
                              name=f"comb_{t}")

            # weave slots: L3(t-2)@1, L1(t-1)@1/3, L4(t-2)@2/3, L2(t-1)@end
            n_mm = 3 * nch
            weave = {}
            if prev2:
                weave[1] = prev2[0]
                weave[(5 * n_mm) // 9] = prev2[1]
            if prev1:
                weave[(2 * n_mm) // 9] = prev1[0]
                weave[(11 * n_mm) // 12] = prev1[1]
            i_mm = 0
            for j in range(3):
                for c in range(nch):
                    nc.tensor.matmul(
                        pst[j][:, c * CHUNK_P: (c + 1) * CHUNK_P],
                        lhsT=xhi[:, c * D + DCH[j]: c * D + DCH[j] + DCWS[j]],
                        rhs=s_hi[:],
                        start=True, stop=True)
                    if i_mm in weave:
                        weave[i_mm]()
                    i_mm += 1
                # this D-chunk's psum is complete: fold in solvent now
                w = DCWS[j]
                nc.vector.tensor_add(
                    out=comb[0:w, j * TILE_P: j * TILE_P + pw],
                    in0=pst[j][:, 0:pw],
                    in1=svt[0:w, j * TILE_P: j * TILE_P + pw])

            prev2 = prev1[2:4] if prev1 else None
            cb = comb
            prev1 = mlp_steps(t, (lambda cb=cb: cb), pw)

        # drain the pipeline tail
        if prev2:
            prev2[0]()
            prev2[1]()
        for step in prev1:
            step()

        # ---- one store for the whole core
        nc.sync.dma_start(aps["out"], out_sb[:])


def build_bass():
    _import_concourse()
    import concourse.mybir as mybir
    import concourse.tile as tile
    from concourse import bacc

    f32 = mybir.dt.float32
    fp16 = mybir.dt.float16
    nc = bacc.Bacc("TRN2", target_bir_lowering=False, debug=False,
                   enable_asserts=False, num_devices=N_CORES)
    aps = {
        "xhi": nc.dram_tensor("xhi", (CHUNK_M, N_CHUNKS * D), fp16,
                              kind="ExternalInput").ap(),
        "svt": nc.dram_tensor("svt", (N_TILES, 128, 3 * TILE_P), fp16,
                              kind="ExternalInput").ap(),
        "s_hi": nc.dram_tensor("s_hi", (CHUNK_M, CHUNK_P), fp16,
                               kind="ExternalInput").ap(),
        "w1p": nc.dram_tensor("w1p", (3, 128, H), fp16, kind="ExternalInput").ap(),
        "w2": nc.dram_tensor("w2", (H, H), fp16, kind="ExternalInput").ap(),
        "w3": nc.dram_tensor("w3", (H, H), fp16, kind="ExternalInput").ap(),
        "w4": nc.dram_tensor("w4", (H, OUT), fp16, kind="ExternalInput").ap(),
        "b1": nc.dram_tensor("b1", (H, 1), f32, kind="ExternalInput").ap(),
        "b2": nc.dram_tensor("b2", (H, 1), f32, kind="ExternalInput").ap(),
        "b3": nc.dram_tensor("b3", (H, 1), f32, kind="ExternalInput").ap(),
        "b4": nc.dram_tensor("b4", (OUT, 1), f32, kind="ExternalInput").ap(),
        "out": nc.dram_tensor("out", (OUT, P_SLOTS), f32,
                              kind="ExternalOutput").ap(),
    }
    with tile.TileContext(nc) as tc:
        emit_program(tc, aps, N_FULL_TILES, PARTIAL_CHUNKS)
    nc.compile()
    return nc


def make_weight_inputs(w1, b1, w2, b2, w3, b3, w4, b4):
    f16 = np.float16
    w1p = np.zeros((3, 128, H), dtype=np.float32)
    for j in range(3):
        w1p[j, 0: DCWS[j], :] = w1[DCH[j]: DCH[j] + DCWS[j], :]
    w4p = np.zeros((128, OUT), np.float32)
    w4p[0:H] = w4
    return {
        "s_hi": build_smat(f16, 1.0),
        "w1p": w1p.astype(f16),
        "w2": np.ascontiguousarray(w2).astype(f16),
        "w3": np.ascontiguousarray(w3).astype(f16),
        "w4": w4p.astype(f16),
        "b1": np.ascontiguousarray(b1, np.float32).reshape(H, 1),
        "b2": np.ascontiguousarray(b2, np.float32).reshape(H, 1),
        "b3": np.ascontiguousarray(b3, np.float32).reshape(H, 1),
        "b4": np.ascontiguousarray(b4, np.float32).reshape(OUT, 1),
    }


def _numpy_reference(mono, solv, seg, w1, b1, w2, b2, w3, b3, w4, b4):
    """Generic fallback: exact math on host for any sorted seg ids."""
    P = solv.shape[0]
    counts = np.bincount(seg, minlength=P).astype(np.float32)
    starts = np.searchsorted(seg, np.arange(P), side="left")
    sums = np.add.reduceat(mono, starts, axis=0)
    sums[counts == 0] = 0.0
    mean = sums / np.maximum(counts, 1.0)[:, None]
    comb = mean + solv
    h = np.maximum(comb @ w1 + b1, 0.0)
    h = np.maximum(h @ w2 + b2, 0.0)
    h = np.maximum(h @ w3 + b3, 0.0)
    return (h @ w4 + b4).astype(np.float32)


_CACHED_NC = None
last_results = None  # BassKernelResults from the most recent device run


def kernel(monomer_features, solvent_features, monomer_seg_ids,
           w1, b1, w2, b2, w3, b3, w4, b4):
    global _CACHED_NC, last_results

    mono = np.ascontiguousarray(monomer_features, dtype=np.float32)
    solv = np.ascontiguousarray(solvent_features, dtype=np.float32)
    seg = np.asarray(monomer_seg_ids).astype(np.int64)
    w1 = np.ascontiguousarray(w1, dtype=np.float32)
    w2 = np.ascontiguousarray(w2, dtype=np.float32)
    w3 = np.ascontiguousarray(w3, dtype=np.float32)
    w4 = np.ascontiguousarray(w4, dtype=np.float32)
    b1 = np.asarray(b1, dtype=np.float32)
    b2 = np.asarray(b2, dtype=np.float32)
    b3 = np.asarray(b3, dtype=np.float32)
    b4 = np.asarray(b4, dtype=np.float32)

    P = solv.shape[0]
    fast = (
        P == P_TOT
        and mono.shape == (N_MONO, D)
        and seg.shape == (N_MONO,)
        and w1.shape == (D, H)
        and np.array_equal(
            seg, np.repeat(np.arange(P_TOT, dtype=np.int64),
                           2 + (np.arange(P_TOT) % 3)))
    )
    if not fast:
        return _numpy_reference(mono, solv, seg, w1, b1, w2, b2, w3, b3, w4, b4)

    _import_concourse()
    from concourse.bass_utils import run_bass_kernel_spmd

    if _CACHED_NC is None:
        _CACHED_NC = build_bass()
    nc = _CACHED_NC

    f16 = np.float16

    # pre-scale monomers by 1/count; fp16 single stream
    counts = (2 + (np.arange(P_TOT) % 3)).astype(np.float32)
    inv = (1.0 / counts)[np.repeat(np.arange(P_TOT), counts.astype(np.int64))]
    m_hi = (mono * inv[:, None]).astype(f16)
    s_hi16 = solv.astype(f16)

    wmaps = make_weight_inputs(w1, b1, w2, b2, w3, b3, w4, b4)

    in_maps = []
    for c in range(N_CORES):
        p0 = CORE_P * c
        p1 = min(CORE_P * (c + 1), P_TOT)
        m0 = 3 * p0
        m1 = m0 + int(np.sum(counts[p0:p1]))
        im = {}
        # monomer stream: [126, N_CHUNKS*300], chunk-major columns
        buf = np.zeros((M_PAD, D), dtype=f16)
        buf[0: m1 - m0] = m_hi[m0:m1]
        im["xhi"] = np.ascontiguousarray(
            buf.reshape(N_CHUNKS, CHUNK_M, D).transpose(1, 0, 2)
            .reshape(CHUNK_M, N_CHUNKS * D))
        # solvent transposed tiles: [25, 128, 3*504] fp16
        buf = np.zeros((P_SLOTS, D), dtype=f16)
        buf[0: p1 - p0] = s_hi16[p0:p1]
        v = buf.reshape(N_TILES, TILE_P, D)
        sv = np.zeros((N_TILES, 128, 3, TILE_P), dtype=f16)
        for j in range(3):
            w = DCWS[j]
            sv[:, 0:w, j, :] = v[:, :, DCH[j]: DCH[j] + w].transpose(0, 2, 1)
        im["svt"] = np.ascontiguousarray(sv.reshape(N_TILES, 128, 3 * TILE_P))
        in_maps.append({**im, **wmaps})

    res = run_bass_kernel_spmd(nc, in_maps, core_ids=list(range(N_CORES)))
    last_results = res

    out = np.empty((P_TOT, OUT), dtype=np.float32)
    for c in range(N_CORES):
        p0 = CORE_P * c
        p1 = min(CORE_P * (c + 1), P_TOT)
        out[p0:p1] = res.results[c]["out"][:, 0: p1 - p0].T
    return out


# revision 22
# speedup vs baseline: 1.1045x; 1.1045x over previous
# Trainium2 Bass kernel for nn_JustMPNN (segment-mean + 4-layer MLP).
#
# Math: per polymer p, mean_p = mean of its monomer rows (counts cycle 2,3,4);
#       combined = mean + solvent;  out = relu-MLP(combined) @ w4 + b4.
#
# Strategy (data-parallel over polymers, 8 cores), fp16 throughout:
#   - Monomers pre-scaled by 1/count on host, cast fp16 (2 B/elt HBM).
#   - chunk = 42 polymers = 14 triples = 126 monomer rows exactly; one
#     0/1 scatter matrix S [126,42] shared by all chunks. matmul(lhsT=
#     x_chunk[:, dslice], rhs=S) gives meanT[d, p] in PSUM, transposed
#     orientation, f32 accumulation.
#   - Solvent bypasses the PE: host packs solvT [d, p] tiles fp16; one
#     DVE tensor_add per D-chunk fuses the PSUM->SBUF copy with the
#     solvent add, emitting combT fp16.
#   - MLP all-fp16 (1 cyc/row), ACT relu+bias straight out of PSUM with
#     fp16 output. Final layer emits outT [7, tile] via lhsT=w4 (7-col
#     weight load); DVE adds b4 during PSUM->SBUF; one [7, 12600] store
#     at the end; host transposes.
#   - Accuracy (HW-verified + bit-exact host sim): scale-relative absmax
#     ~7e-4, l2rel ~5.4e-4 -- ~28x under the 2e-2 gate.
#
# Host-side packing re-lays every stream so each per-tile DMA is a
# [partitions, contiguous-bytes] transfer (descriptor-efficient).

import os
import sys

import numpy as np

# ---------------------------------------------------------------- constants
P_TOT = 100000
D = 300
H = 128
OUT = 7
N_MONO = 299999

CHUNK_P = 42           # polymers per chunk (14 triples)
CHUNK_M = 126          # monomer rows per chunk
TILE_CHUNKS = 12       # chunks per MLP tile
TILE_P = CHUNK_P * TILE_CHUNKS  # 504

N_CORES = 8
CORE_P = 12501                  # polymers per core (cores 0-6); core 7: 12493
N_CHUNKS = 298                  # ceil(12501/42) -> 12516 padded polys
P_PAD = N_CHUNKS * CHUNK_P      # 12516
N_FULL_TILES = 24               # 24*504 = 12096
PARTIAL_CHUNKS = N_CHUNKS - N_FULL_TILES * TILE_CHUNKS  # 10 chunks = 420
N_TILES = N_FULL_TILES + 1      # 25
P_SLOTS = N_TILES * TILE_P      # 12600 (uniform 504-wide out/solv slots)
M_PAD = N_CHUNKS * CHUNK_M      # 37548

DCH = (0, 128, 256)    # D-chunk offsets
DCWS = (128, 128, 44)  # D-chunk widths


def _import_concourse():
    for p in ("/opt/trn_rl_repo", "/root/.axon_site/_ro/trn_rl_repo"):
        if os.path.isdir(p) and p not in sys.path:
            sys.path.insert(0, p)


def build_smat(dtype, val):
    """S [CHUNK_M, CHUNK_P]: S[r,p] = val for monomer rows of polymer p."""
    s = np.zeros((CHUNK_M, CHUNK_P), dtype=np.float32)
    r = 0
    for p in range(CHUNK_P):
        cnt = 2 + (p % 3)
        s[r: r + cnt, p] = val
        r += cnt
    assert r == CHUNK_M
    return s.astype(dtype)


def emit_program(tc, aps, n_full_tiles, partial_chunks):
    from contextlib import ExitStack

    import concourse.mybir as mybir

    nc = tc.nc
    f32 = mybir.dt.float32
    fp16 = mybir.dt.float16
    Relu = mybir.ActivationFunctionType.Relu
    Add = mybir.AluOpType.add
    Bypass = mybir.AluOpType.bypass

    n_tiles = n_full_tiles + (1 if partial_chunks else 0)

    with ExitStack() as ctx:
        consts = ctx.enter_context(tc.tile_pool(name="consts", bufs=1))
        xpool = ctx.enter_context(tc.tile_pool(name="xp", bufs=3))
        svpool = ctx.enter_context(tc.tile_pool(name="svp", bufs=2))
        cpool = ctx.enter_context(tc.tile_pool(name="cp", bufs=2))
        hpool = ctx.enter_context(tc.tile_pool(name="hp", bufs=2))
        opool = ctx.enter_context(tc.tile_pool(name="op", bufs=1))
        ps1 = ctx.enter_context(tc.tile_pool(name="ps1", bufs=1, space="PSUM"))
        psh = ctx.enter_context(tc.tile_pool(name="psh", bufs=2, space="PSUM"))
        ps4 = ctx.enter_context(tc.tile_pool(name="ps4", bufs=2, space="PSUM"))

        # ---- tile-0 data first (the critical path), then constants
        xhi0 = xpool.tile([CHUNK_M, TILE_CHUNKS * D], fp16, tag="xhi",
                          name="xhi_0")
        nc.sync.dma_start(xhi0[:], aps["xhi"][:, 0: TILE_CHUNKS * D])
        svt0 = svpool.tile([128, 3 * TILE_P], fp16, tag="svt", name="svt_0")
        nc.sync.dma_start(svt0[:], aps["svt"][0])

        s_hi = consts.tile([CHUNK_M, CHUNK_P], fp16)
        nc.sync.dma_start(s_hi[:], aps["s_hi"])

        w1 = consts.tile([128, 3, H], fp16)
        nc.sync.dma_start(w1[:], aps["w1p"].rearrange("j k m -> k j m"))
        w2 = consts.tile([128, H], fp16)
        nc.sync.dma_start(w2[:], aps["w2"])
        w3 = consts.tile([128, H], fp16)
        nc.sync.dma_start(w3[:], aps["w3"])
        w4 = consts.tile([128, OUT], fp16)
        nc.sync.dma_start(w4[:], aps["w4"])
        b1 = consts.tile([128, 1], f32)
        nc.sync.dma_start(b1[:], aps["b1"])
        b2 = consts.tile([128, 1], f32)
        nc.sync.dma_start(b2[:], aps["b2"])
        b3 = consts.tile([128, 1], f32)
        nc.sync.dma_start(b3[:], aps["b3"])
        b4 = consts.tile([OUT, 1], f32)
        nc.sync.dma_start(b4[:], aps["b4"])

        # whole core's output stays in SBUF as outT [7, P_SLOTS]
        out_sb = opool.tile([OUT, P_SLOTS], f32)

        xhi_d = aps["xhi"]
        sv_d = aps["svt"]

        # Tile t's stage-1 runs j-outer so each D-chunk's DVE add (psum +
        # solvT -> combT fp16) fires as soon as that psum completes; tile
        # t-1's MLP layers are woven between stage-1 matmuls at fixed flat
        # indices so the PE never idles long enough to re-throttle.
        def mlp_steps(t, comb_fn, pw):
            state = {}

            def lyr(li, rhs_fn, whi, bias, nk=1):
                def go():
                    rhs = rhs_fn()
                    ph = psh.tile([128, TILE_P], f32, tag="psh",
                                  name=f"ph_{t}_{li}")
                    for k in range(nk):
                        kw = DCWS[k] if nk == 3 else 128
                        lh = whi[0:kw, k, :] if nk == 3 else whi[:]
                        rh = (rhs[0:kw, k * TILE_P: k * TILE_P + pw]
                              if nk == 3 else rhs[:, 0:pw])
                        nc.tensor.matmul(ph[:, 0:pw], lhsT=lh, rhs=rh,
                                         start=(k == 0), stop=(k == nk - 1))
                    hh = hpool.tile([128, TILE_P], fp16, tag=f"hh{li}",
                                    name=f"hh_{t}_{li}")
                    nc.scalar.activation(hh[:, 0:pw], ph[:, 0:pw], Relu,
                                         bias=bias[:, 0:1])
                    state[li] = hh
                return go

            def l4():
                h3 = state[3]
                po = ps4.tile([OUT, TILE_P], f32, tag="ps4", name=f"po_{t}")
                nc.tensor.matmul(po[:, 0:pw], lhsT=w4[:, 0:OUT],
                                 rhs=h3[:, 0:pw], start=True, stop=True)
                nc.vector.tensor_scalar(
                    out=out_sb[:, t * TILE_P: t * TILE_P + pw],
                    in0=po[:, 0:pw], scalar1=b4[:, 0:1], op0=Add,
                    scalar2=None, op1=Bypass)

            return [lyr(1, comb_fn, w1, b1, nk=3),
                    lyr(2, lambda: state[1], w2, b2),
                    lyr(3, lambda: state[2], w3, b3),
                    l4]

        # Two-pass MLP pipeline: tile p's L1/L2 are woven into stage-1 of
        # pass p+1 and L3/L4 into pass p+2, so each layer's ACT latency
        # hides under a full stage-1 matmul stream and the PE stays dense.
        prev1 = None  # [L1, L2, L3, L4] thunks of tile t-1
        prev2 = None  # [L3, L4] thunks of tile t-2
        for t in range(n_tiles):
            nch = TILE_CHUNKS if t < n_full_tiles else partial_chunks
            pw = nch * CHUNK_P

            # ---- loads (per-partition contiguous); tile 0 preloaded
            if t == 0:
                xhi, svt = xhi0, svt0
            else:
                xhi = xpool.tile([CHUNK_M, TILE_CHUNKS * D], fp16, tag="xhi",
                                 name=f"xhi_{t}")
                nc.sync.dma_start(
                    xhi[:, 0: nch * D],
                    xhi_d[:, t * TILE_CHUNKS * D:
                          t * TILE_CHUNKS * D + nch * D])
                svt = svpool.tile([128, 3 * TILE_P], fp16, tag="svt",
                                  name=f"svt_{t}")
                nc.sync.dma_start(svt[:], sv_d[t])

            pst = [ps1.tile([DCWS[j], TILE_P], f32, tag=f"s1_{j}",
                            name=f"s1_{j}_{t}") for j in range(3)]
            comb = cpool.tile([128, 3 * TILE_P], fp16, tag="comb",
                              name=f"comb_{t}")

            # weave slots: L3(t-2)@1, L1(t-1)@1/3, L4(t-2)@2/3, L2(t-1)@end
            n_mm = 3 * nch
            weave = {}
            if prev2:
                weave[1] = prev2[0]
                weave[(2 * n_mm) // 3] = prev2[1]
            if prev1:
                weave[n_mm // 3] = prev1[0]
                weave[n_mm - 1] = prev1[1]
            i_mm = 0
            for j in range(3):
                for c in range(nch):
                    nc.tensor.matmul(
                        pst[j][:, c * CHUNK_P: (c + 1) * CHUNK_P],
                        lhsT=xhi[:, c * D + DCH[j]: c * D + DCH[j] + DCWS[j]],
                        rhs=s_hi[:],
                        start=True, stop=True)
                    if i_mm in weave:
                        weave[i_mm]()
                    i_mm += 1
                # this D-chunk's psum is complete: fold in solvent now
                w = DCWS[j]
                nc.vector.tensor_add(
                    out=comb[0:w, j * TILE_P: j * TILE_P + pw],
                    in0=pst[j][:, 0:pw],
                    in1=svt[0:w, j * TILE_P: j * TILE_P + pw])

            prev2 = prev1[2:4] if prev1 else None
            cb = comb
            prev1 = mlp_steps(t, (lambda cb=cb: cb), pw)

        # drain the pipeline tail
        if prev2:
            prev2[0]()
            prev2[1]()
        for step in prev1:
            step()

        # ---- one store for the whole core
        nc.sync.dma_start(aps["out"], out_sb[:])


def build_bass():
    _import_concourse()
    import concourse.mybir as mybir
    import concourse.tile as tile
    from concourse import bacc

    f32 = mybir.dt.float32
    fp16 = mybir.dt.float16
    nc = bacc.Bacc("TRN2", target_bir_lowering=False, debug=False,
                   enable_asserts=False, num_devices=N_CORES)
    aps = {
        "xhi": nc.dram_tensor("xhi", (CHUNK_M, N_CHUNKS * D), fp16,
                              kind="ExternalInput").ap(),
        "svt": nc.dram_tensor("svt", (N_TILES, 128, 3 * TILE_P), fp16,
                              kind="ExternalInput").ap(),
        "s_hi": nc.dram_tensor("s_hi", (CHUNK_M, CHUNK_P), fp16,
                               kind="ExternalInput").ap(),
        "w1p": nc.dram_tensor("w1p", (3, 128, H), fp16, kind="ExternalInput").ap(),
        "w2": nc.dram_tensor("w2", (H, H), fp16, kind="ExternalInput").ap(),
        "w3": nc.dram_tensor("w3", (H, H), fp16, kind="ExternalInput").ap(),
        "w4": nc.dram_tensor("w4", (H, OUT), fp16, kind="ExternalInput").ap(),
        "b1": nc.dram_tensor("b1", (H, 1), f32, kind="ExternalInput").ap(),
        "b2": nc.dram_tensor("b2", (H, 1), f32, kind="ExternalInput").ap(),
        "b3": nc.dram_tensor("b3", (H, 1), f32, kind="ExternalInput").ap(),
        "b4": nc.dram_tensor("b4", (OUT, 1), f32, kind="ExternalInput").ap(),
        "out": nc.dram_tensor("out", (OUT, P_SLOTS), f32,
                              kind="ExternalOutput").ap(),
    }
    with tile.TileContext(nc) as tc:
        emit_program(tc, aps, N_FULL_TILES, PARTIAL_CHUNKS)
    nc.compile()
    return nc


def make_weight_inputs(w1, b1, w2, b2, w3, b3, w4, b4):
    f16 = np.float16
    w1p = np.zeros((3, 128, H), dtype=np.float32)
    for j in range(3):
        w1p[j, 0: DCWS[j], :] = w1[DCH[j]: DCH[j] + DCWS[j], :]
    w4p = np.zeros((128, OUT), np.float32)
    w4p[0:H] = w4
    return {
        "s_hi": build_smat(f16, 1.0),
        "w1p": w1p.astype(f16),
        "w2": np.ascontiguousarray(w2).astype(f16),
        "w3": np.ascontiguousarray(w3).astype(f16),
        "w4": w4p.astype(f16),
        "b1": np.ascontiguousarray(b1, np.float32).reshape(H, 1),
        "b2": np.ascontiguousarray(b2, np.float32).reshape(H, 1),
        "b3": np.ascontiguousarray(b3, np.float32).reshape(H, 1),
        "b4": np.ascontiguousarray(b4, np.float32).reshape(OUT, 1),
    }


def _numpy_reference(mono, solv, seg, w1, b1, w2, b2, w3, b3, w4, b4):
    """Generic fallback: exact math on host for any sorted seg ids."""
    P = solv.shape[0]
    counts = np.bincount(seg, minlength=P).astype(np.float32)
    starts = np.searchsorted(seg, np.arange(P), side="left")
    sums = np.add.reduceat(mono, starts, axis=0)
    sums[counts == 0] = 0.0
    mean = sums / np.maximum(counts, 1.0)[:, None]
    comb = mean + solv
    h = np.maximum(comb @ w1 + b1, 0.0)
    h = np.maximum(h @ w2 + b2, 0.0)
    h = np.maximum(h @ w3 + b3, 0.0)
    return (h @ w4 + b4).astype(np.float32)


_CACHED_NC = None
last_results = None  # BassKernelResults from the most recent device run


def kernel(monomer_features, solvent_features, monomer_seg_ids,
           w1, b1, w2, b2, w3, b3, w4, b4):
    global _CACHED_NC, last_results

    mono = np.ascontiguousarray(monomer_features, dtype=np.float32)
    solv = np.ascontiguousarray(solvent_features, dtype=np.float32)
    seg = np.asarray(monomer_seg_ids).astype(np.int64)
    w1 = np.ascontiguousarray(w1, dtype=np.float32)
    w2 = np.ascontiguousarray(w2, dtype=np.float32)
    w3 = np.ascontiguousarray(w3, dtype=np.float32)
    w4 = np.ascontiguousarray(w4, dtype=np.float32)
    b1 = np.asarray(b1, dtype=np.float32)
    b2 = np.asarray(b2, dtype=np.float32)
    b3 = np.asarray(b3, dtype=np.float32)
    b4 = np.asarray(b4, dtype=np.float32)

    P = solv.shape[0]
    fast = (
        P == P_TOT
        and mono.shape == (N_MONO, D)
        and seg.shape == (N_MONO,)
        and w1.shape == (D, H)
        and np.array_equal(
            seg, np.repeat(np.arange(P_TOT, dtype=np.int64),
                           2 + (np.arange(P_TOT) % 3)))
    )
    if not fast:
        return _numpy_reference(mono, solv, seg, w1, b1, w2, b2, w3, b3, w4, b4)

    _import_concourse()
    from concourse.bass_utils import run_bass_kernel_spmd

    if _CACHED_NC is None:
        _CACHED_NC = build_bass()
    nc = _CACHED_NC

    f16 = np.float16

    # pre-scale monomers by 1/count; fp16 single stream
    counts = (2 + (np.arange(P_TOT) % 3)).astype(np.float32)
    inv = (1.0 / counts)[np.repeat(np.arange(P_TOT), counts.astype(np.int64))]
    m_hi = (mono * inv[:, None]).astype(f16)
    s_hi16 = solv.astype(f16)

    wmaps = make_weight_inputs(w1, b1, w2, b2, w3, b3, w4, b4)

    in_maps = []
    for c in range(N_CORES):
        p0 = CORE_P * c
        p1 = min(CORE_P * (c + 1), P_TOT)
        m0 = 3 * p0
        m1 = m0 + int(np.sum(counts[p0:p1]))
        im = {}
        # monomer stream: [126, N_CHUNKS*300], chunk-major columns
        buf = np.zeros((M_PAD, D), dtype=f16)
        buf[0: m1 - m0] = m_hi[m0:m1]
        im["xhi"] = np.ascontiguousarray(
            buf.reshape(N_CHUNKS, CHUNK_M, D).transpose(1, 0, 2)
            .reshape(CHUNK_M, N_CHUNKS * D))
        # solvent transposed tiles: [25, 128, 3*504] fp16
        buf = np.zeros((P_SLOTS, D), dtype=f16)
        buf[0: p1 - p0] = s_hi16[p0:p1]
        v = buf.reshape(N_TILES, TILE_P, D)
        sv = np.zeros((N_TILES, 128, 3, TILE_P), dtype=f16)
        for j in range(3):
            w = DCWS[j]
            sv[:, 0:w, j, :] = v[:, :, DCH[j]: DCH[j] + w].transpose(0, 2, 1)
        im["svt"] = np.ascontiguousarray(sv.reshape(N_TILES, 128, 3 * TILE_P))
        in_maps.append({**im, **wmaps})

    res = run_bass_kernel_spmd(nc, in_maps, core_ids=list(range(N_CORES)))
    last_results = res

    out = np.empty((P_TOT, OUT), dtype=np.float32)
    for c in range(N_CORES):
        p0 = CORE_P * c
        p1 = min(CORE_P * (c + 1), P_TOT)
        out[p0:p1] = res.results[c]["out"][:, 0: p1 - p0].T
    return out
